# revision 1
# baseline (speedup 1.0000x reference)
"""Trainium2 Bass kernel for BinarizeConv2dSDP.

Math (reference):
    s   = M + rv @ Z          (the rsqrt normalization is sign-preserving:
                               w = (m + rv@z) * rsqrt(...) with rsqrt > 0,
                               so sign(w) == sign(s))
    bw  = sign(s)             (O, I, 3, 3)
    ba  = sign(x)             (B, C, H, W)
    out = conv2d(ba, bw, stride 1, pad 1) * Alpha

Strategy:
    - Data-parallel over batch: 8 cores x 4 images each. M/Z/Alpha replicated.
    - Weight synthesis on-device per core: 5 fused (z*rv_k)+prev ops; each
      full-width op (~1.4us) pipelines behind the per-Z DMA wire (~1.7us),
      then sign -> 9 PE transposes -> 2 packing copies.
    - Binarized conv: sign(x) stored fp8e4 in a zero-padded [128, 58 x 64]
      SBUF image (row stride 64 so a vertical tap pair is a 64B step).
      Per output row-block, 3 DoubleRow matmuls (vertical tap pairs, K=256)
      + 3 normal fp8 matmuls (ky=2 taps) accumulate into PSUM. +-1 is exact
      in fp8e4/bf16 and PSUM accumulates in f32, so results are exact.
    - All input DMAs are issued up front (x0 between the weight loads);
      output stores follow on the same queue, sem-gated per evacuation.
    - Alpha applied during PSUM->SBUF evacuation; f32 out. Bit-equal to the
      reference modulo conv summation order (integer-exact).
"""

import os
import numpy as np

import concourse.bass as bass
import concourse.tile as tile
from concourse import bacc, mybir
from concourse.bass_utils import run_bass_kernel_spmd
from concourse.masks import make_identity

F32 = mybir.dt.float32
BF16 = mybir.dt.bfloat16
FP8 = mybir.dt.float8e4

USE_FP8 = bool(int(os.environ.get("BASS_KERNEL_FP8", "1")))

B_FULL = 32
N_CORES = 8
B_CORE = B_FULL // N_CORES  # 4 images per core
C = 128      # in channels
O = 128      # out channels
H = W = 56
HP = 58                      # padded rows
WP = 64 if USE_FP8 else 58   # padded row stride (64 -> tap-pair step is 64B)
KS = 3
NTAPS = KS * KS
IKK = C * NTAPS  # 1152
ROWS_PER_TILE = 8           # output rows per PSUM tile -> N = 8*56 = 448
N_TILE = ROWS_PER_TILE * W  # 448 fp32 <= 512 (one PSUM bank)
N_ROW_TILES = H // ROWS_PER_TILE  # 7
ADT = FP8 if USE_FP8 else BF16


def build_program(rv: np.ndarray, n_img: int = B_CORE):
    """Build the per-core Bass program. rv values are baked as immediates."""
    nc = bacc.Bacc(
        "TRN2",
        target_bir_lowering=False,
        debug=False,
        num_devices=N_CORES,
    )

    x_t = nc.dram_tensor("x", (n_img, C, H, W), F32, kind="ExternalInput").ap()
    a_t = nc.dram_tensor("Alpha", (O, 1, 1), F32, kind="ExternalInput").ap()
    m_t = nc.dram_tensor("M", (O, C, KS, KS), F32, kind="ExternalInput").ap()
    z_t = nc.dram_tensor("Z", (5, O, C, KS, KS), F32, kind="ExternalInput").ap()
    out_t = nc.dram_tensor("out", (n_img, O, H, W), F32, kind="ExternalOutput").ap()

    rv = np.asarray(rv, dtype=np.float32).reshape(-1)
    assert rv.shape[0] == 5

    with tile.TileContext(nc) as tc:
        with (
            tc.tile_pool(name="const", bufs=1) as const_pool,
            tc.tile_pool(name="wsyn", bufs=1) as wsyn_pool,
            tc.tile_pool(name="imgs", bufs=1) as img_pool,
            tc.tile_pool(name="xstage", bufs=4) as x_pool,
            tc.tile_pool(name="evac", bufs=8) as ev_pool,
            tc.tile_pool(name="cpsum", bufs=6, space="PSUM") as cpsum_pool,
            tc.tile_pool(name="tpsum", bufs=1, space="PSUM") as tpsum_pool,
        ):
            # --- x0 first on the wire: its sign hides under the Z DMAs ---
            alpha_sb = const_pool.tile([O, 1], F32)
            nc.sync.dma_start(alpha_sb, a_t.rearrange("o a b -> o (a b)"))
            x_tiles = [None] * n_img
            x_tiles[0] = x_pool.tile([C, H * W], F32, name="x0", tag="xin")
            nc.sync.dma_start(x_tiles[0], x_t[0].rearrange("c h w -> c (h w)"))

            identity = const_pool.tile([128, 128], BF16)
            make_identity(nc, identity)

            m_sb = wsyn_pool.tile([O, IKK], F32)
            nc.sync.dma_start(m_sb, m_t.rearrange("o i kh kw -> o (i kh kw)"))
            z_sbs = []
            for k in range(5):
                z_sb = wsyn_pool.tile([O, IKK], F32, name=f"z{k}", tag=f"z{k}")
                nc.sync.dma_start(
                    z_sb, z_t[k].rearrange("o i kh kw -> o (i kh kw)")
                )
                z_sbs.append(z_sb)

            # --- per-image padded sign(x) buffers (borders zeroed once) ---
            padded = []
            for img in range(n_img):
                pd = img_pool.tile(
                    [C, HP * WP], ADT, name=f"pad{img}", tag=f"pad{img}"
                )
                pd3 = pd.rearrange("p (h w) -> p h w", w=WP)
                nc.gpsimd.memset(pd3[:, 0, 0:HP], 0.0)
                nc.gpsimd.memset(pd3[:, HP - 1, 0:HP], 0.0)
                nc.gpsimd.memset(pd3[:, 1 : HP - 1, 0:1], 0.0)
                nc.gpsimd.memset(pd3[:, 1 : HP - 1, HP - 1 : HP], 0.0)
                padded.append(pd3)

            def sign_image(img):
                pd3 = padded[img]
                nc.scalar.sign(
                    pd3[:, 1 : 1 + H, 1 : 1 + W],
                    x_tiles[img].rearrange("c (h w) -> c h w", w=W),
                )

            sign_image(0)

            # --- weight synthesis: s = M + sum_k rv_k Z_k.
            # The tail after Z4 lands is chunked over the free (i) dim so
            # sign/transposes of earlier chunks overlap the last stt ops;
            # every op still spans all 128 partitions (full engine lanes).
            NCHUNK, CCH = 4, 32
            s_sb = wsyn_pool.tile([O, IKK], F32)
            bw_nat = wsyn_pool.tile([O, IKK], BF16)
            bw3 = bw_nat.rearrange("o (i t) -> o i t", t=NTAPS)
            if USE_FP8:
                bw_pair = wsyn_pool.tile([C, KS, 2, O], FP8)
                bw_single = wsyn_pool.tile([C, KS, O], FP8)
                tpP = tpsum_pool.tile([128, KS * 2 * O], BF16)
                tpS = tpsum_pool.tile([128, KS * O], BF16)
            else:
                bw_lhsT = wsyn_pool.tile([C, NTAPS, O], BF16)
                tpP = tpsum_pool.tile([128, 4 * O], BF16)
                tpS = tpsum_pool.tile([128, 5 * O], BF16)
            for ic in range(NCHUNK):
                csl = slice(ic * CCH * NTAPS, (ic + 1) * CCH * NTAPS)
                for k in range(5):
                    nc.vector.scalar_tensor_tensor(
                        out=s_sb[:, csl],
                        in0=z_sbs[k][:, csl],
                        scalar=float(rv[k]),
                        in1=(m_sb if k == 0 else s_sb)[:, csl],
                        op0=mybir.AluOpType.mult,
                        op1=mybir.AluOpType.add,
                    )
                nc.scalar.sign(bw_nat[:, csl], s_sb[:, csl])
                psl = slice(ic * CCH, (ic + 1) * CCH)
                for t in range(NTAPS):
                    ky, kx = divmod(t, KS)
                    if USE_FP8:
                        dst, toff = (
                            (tpP, (kx * 2 + ky) * O) if ky < 2 else (tpS, kx * O)
                        )
                    else:
                        dst, toff = (tpP, t * O) if t < 4 else (tpS, (t - 4) * O)
                    nc.tensor.transpose(
                        dst[psl, toff : toff + O],
                        bw3[:, psl, t],
                        identity,
                        tile_position=(0, ic * CCH),
                    )
            if USE_FP8:
                nc.scalar.copy(
                    bw_pair.rearrange("p a b o -> p (a b o)"), tpP
                )
                nc.vector.tensor_copy(
                    bw_single.rearrange("p a o -> p (a o)"), tpS
                )
            else:
                nc.vector.tensor_copy(
                    bw_lhsT[:, 0:4, :],
                    tpP.rearrange("p (t o) -> p t o", o=O),
                )
                nc.vector.tensor_copy(
                    bw_lhsT[:, 4:NTAPS, :],
                    tpS.rearrange("p (t o) -> p t o", o=O),
                )

            # --- main conv loop; next image's load+sign emitted before this
            # image's tiles so ACT never head-of-line blocks the sign ---
            for img in range(n_img):
                if img + 1 < n_img:
                    nxt = img + 1
                    x_tiles[nxt] = x_pool.tile(
                        [C, H * W], F32, name=f"x{nxt}", tag="xin"
                    )
                    nc.sync.dma_start(
                        x_tiles[nxt], x_t[nxt].rearrange("c h w -> c (h w)")
                    )
                    sign_image(nxt)
                pd3 = padded[img]

                for nt in range(N_ROW_TILES):
                    y0 = nt * ROWS_PER_TILE
                    cv = cpsum_pool.tile([O, N_TILE], F32, tag="cv")
                    if USE_FP8:
                        for kx in range(KS):
                            win0 = pd3[:, y0 : y0 + ROWS_PER_TILE, kx : kx + W]
                            ap4 = bass.AP(
                                win0.tensor,
                                win0.offset,
                                [list(win0.ap[0]), [WP, 2]]
                                + [list(p) for p in win0.ap[1:]],
                            )
                            nc.tensor.matmul(
                                cv,
                                bw_pair[:, kx],
                                ap4,
                                start=(kx == 0),
                                stop=False,
                                perf_mode=mybir.MatmulPerfMode.DoubleRow,
                            )
                        for kx in range(KS):
                            win = pd3[
                                :, y0 + 2 : y0 + 2 + ROWS_PER_TILE, kx : kx + W
                            ]
                            nc.tensor.matmul(
                                cv,
                                bw_single[:, kx],
                                win,
                                start=False,
                                stop=(kx == KS - 1),
                            )
                    else:
                        t = 0
                        for ky in range(KS):
                            for kx in range(KS):
                                win = pd3[
                                    :,
                                    y0 + ky : y0 + ky + ROWS_PER_TILE,
                                    kx : kx + W,
                                ]
                                nc.tensor.matmul(
                                    cv,
                                    bw_lhsT[:, t, :],
                                    win,
                                    start=(t == 0),
                                    stop=(t == NTAPS - 1),
                                )
                                t += 1
                    ev = ev_pool.tile([O, N_TILE], F32, tag="ev")
                    nc.vector.tensor_scalar_mul(ev, cv, alpha_sb[:, 0:1])
                    # stores on their own queues: never head-of-line block
                    # the x loads riding the sync queue
                    dma_eng = nc.scalar if (nt % 2 == 0) else nc.gpsimd
                    dma_eng.dma_start(
                        out_t[img, :, y0 : y0 + ROWS_PER_TILE, :],
                        ev.rearrange("o (h w) -> o h w", w=W),
                    )

    nc.compile()
    return nc


def _ensure_ntff_hook():
    """Register the axon NTFF profiling hook if the image's antenv lacks it.

    Only used when BASS_KERNEL_TRACE=1 (dev profiling); best-effort.
    """
    import sys
    import types

    try:
        import antenv

        if hasattr(antenv, "axon_hooks"):
            return
        mod = types.ModuleType("antenv.axon_hooks")
        _hook = [None]
        mod.set_axon_ntff_profile_hook = lambda h: _hook.__setitem__(0, h)
        mod.get_axon_ntff_profile_hook = lambda: _hook[0]
        sys.modules["antenv.axon_hooks"] = mod
        antenv.axon_hooks = mod
        from trn_agent_boot.trn_boot import _ntff_profile_via_ctypes

        mod.set_axon_ntff_profile_hook(
            _ntff_profile_via_ctypes("/opt/axon/libaxon_pjrt.so")
        )
    except Exception as e:  # pragma: no cover - profiling is optional
        print(f"NTFF hook registration failed ({e}); tracing disabled")


def kernel(x, Alpha, M, Z, rv):
    x = np.ascontiguousarray(np.asarray(x, dtype=np.float32))
    Alpha = np.ascontiguousarray(np.asarray(Alpha, dtype=np.float32))
    M = np.ascontiguousarray(np.asarray(M, dtype=np.float32))
    Z = np.ascontiguousarray(np.asarray(Z, dtype=np.float32))
    rv = np.asarray(rv, dtype=np.float32)

    trace = bool(int(os.environ.get("BASS_KERNEL_TRACE", "0")))
    if trace:
        _ensure_ntff_hook()

    nc = build_program(rv)

    in_maps = []
    for c in range(N_CORES):
        in_maps.append(
            {
                "x": np.ascontiguousarray(x[c * B_CORE : (c + 1) * B_CORE]),
                "Alpha": Alpha,
                "M": M,
                "Z": Z,
            }
        )

    res = run_bass_kernel_spmd(
        nc,
        in_maps,
        core_ids=list(range(N_CORES)),
        trace=trace,
    )
    out = np.concatenate([res.results[c]["out"] for c in range(N_CORES)], axis=0)
    if trace:
        kernel.last_results = res
    return out



# revision 10
# speedup vs baseline: 1.1036x; 1.1036x over previous
"""Trainium2 Bass kernel for BinarizeConv2dSDP.

Math (reference):
    s   = M + rv @ Z          (the rsqrt normalization is sign-preserving:
                               w = (m + rv@z) * rsqrt(...) with rsqrt > 0,
                               so sign(w) == sign(s))
    bw  = sign(s)             (O, I, 3, 3)
    ba  = sign(x)             (B, C, H, W)
    out = conv2d(ba, bw, stride 1, pad 1) * Alpha

Strategy (v2):
    - Data-parallel over batch: 8 cores x 4 images each. M/Z/Alpha replicated.
    - Weight synthesis on the PE: per 288-col chunk, ACT copies the M chunk
      into PSUM, then 5 fp32r matmuls (rv_k * I as stationary) accumulate
      rv_k Z_k on top at full rate (N >= 256), paced by the Z DMA arrivals;
      ACT signs straight from PSUM. M enters exactly (f32); only Z terms see
      fp32r input rounding -> a handful of borderline sign flips, well under
      the 2e-2 gate. The synth matmuls double as HAM clock-gate warm-up.
    - Binarized conv, 5 passes per 8-row tile (was 6): sign(x) lives in a
      zero-padded [128, 114 x 64] fp8 image; rows 58..113 hold a one-column-
      left-shifted duplicate of rows 2..57. Vertical tap pairs (ky 0+1) use
      DoubleRow with pair step 64; the (2,0)+(2,1) pair uses DoubleRow with
      pair step 56*64=3584 into the duplicate; (2,2) is a single matmul.
      +-1 is exact in fp8/bf16 and PSUM accumulates f32 -> integer-exact.
    - Signs (main + shifted dup) chunked and interleaved on ACT per image so
      conv tiles release as rows land. Evacuation (x Alpha) on DVE; output
      stored fp16 (conv integers <= 1152 are fp16-exact; only the Alpha
      product rounds, ~5e-4), host upcasts. Stores ride the sync queue
      behind the input loads; evac pool is deep enough to absorb that.
"""

import os
import numpy as np

import concourse.bass as bass
import concourse.tile as tile
from concourse import bacc, mybir
from concourse.bass_utils import run_bass_kernel_spmd
from concourse.masks import make_identity

F32 = mybir.dt.float32
F32R = mybir.dt.float32r
F16 = mybir.dt.float16
BF16 = mybir.dt.bfloat16
FP8 = mybir.dt.float8e4

USE_FP8 = bool(int(os.environ.get("BASS_KERNEL_FP8", "1")))
OUT16 = bool(int(os.environ.get("BASS_KERNEL_OUT16", "1")))
N_WARM = int(os.environ.get("BASS_KERNEL_WARM", "12"))

B_FULL = 32
N_CORES = 8
B_CORE = B_FULL // N_CORES  # 4 images per core
C = 128      # in channels
O = 128      # out channels
H = W = 56
HP = 58                      # padded rows
WP = 64 if USE_FP8 else 58   # padded row stride (64 -> vertical pair step 64B)
DUP = 56                     # dup row r stored at padded row r + DUP
HP2 = HP + DUP               # 114 rows total (2..57 duplicated, shifted left 1)
KS = 3
NTAPS = KS * KS
IKK = C * NTAPS  # 1152
ROWS_PER_TILE = 8           # output rows per PSUM tile -> N = 8*56 = 448
N_TILE = ROWS_PER_TILE * W  # 448 fp32 <= 512 (one PSUM bank)
N_ROW_TILES = H // ROWS_PER_TILE  # 7
ADT = FP8 if USE_FP8 else BF16
ODT = F16 if OUT16 else F32

CCH = 288                    # synth column chunk (32 in-channels x 9 taps)
NCHUNK = IKK // CCH          # 4
# x0 arrives in row chunks so sign/conv can start before the full image
X0_CHUNKS = ((0, 16), (16, 36), (36, 56))
XN_CHUNKS = ((0, 28), (28, 56))


def build_program(rv: np.ndarray, n_img: int = B_CORE):
    """Build the per-core Bass program. rv values are baked as immediates."""
    nc = bacc.Bacc(
        "TRN2",
        target_bir_lowering=False,
        debug=False,
        num_devices=N_CORES,
    )

    x_t = nc.dram_tensor("x", (n_img, C, H, W), F32, kind="ExternalInput").ap()
    a_t = nc.dram_tensor("Alpha", (O, 1, 1), F32, kind="ExternalInput").ap()
    m_t = nc.dram_tensor("M", (O, C, KS, KS), F32, kind="ExternalInput").ap()
    z_t = nc.dram_tensor("Z", (5, O, C, KS, KS), F32R, kind="ExternalInput").ap()
    out_t = nc.dram_tensor("out", (n_img, O, H, W), ODT, kind="ExternalOutput").ap()

    rv = np.asarray(rv, dtype=np.float32).reshape(-1)
    assert rv.shape[0] == 5

    with tile.TileContext(nc) as tc:
        with (
            tc.tile_pool(name="const", bufs=1) as const_pool,
            tc.tile_pool(name="wsyn", bufs=1) as wsyn_pool,
            tc.tile_pool(name="imgs", bufs=1) as img_pool,
            tc.tile_pool(name="xstage", bufs=4) as x_pool,
            tc.tile_pool(name="evac", bufs=14) as ev_pool,
            tc.tile_pool(name="cpsum", bufs=4, space="PSUM") as cpsum_pool,
            tc.tile_pool(name="spsum", bufs=1, space="PSUM") as spsum_pool,
            tc.tile_pool(name="tpsum", bufs=1, space="PSUM") as tpsum_pool,
        ):
            # --- head DMAs, all on the sync queue: FIFO order == priority.
            # M first (needed for every synth chunk), then Z halves in
            # synthesis order, then x0 in row chunks, then x1..x3.
            alpha_sb = const_pool.tile([O, 1], F32)
            nc.scalar.dma_start(alpha_sb, a_t.rearrange("o a b -> o (a b)"))

            m_sb = wsyn_pool.tile([O, IKK], F32)
            nc.sync.dma_start(m_sb, m_t.rearrange("o i kh kw -> o (i kh kw)"))
            z_sbs = []
            for k in range(5):
                z_sbs.append(
                    wsyn_pool.tile([O, IKK], F32R, name=f"z{k}", tag=f"z{k}")
                )
            LHALF = slice(0, 3 * CCH)
            RHALF = slice(3 * CCH, IKK)
            for k in range(5):
                nc.sync.dma_start(
                    z_sbs[k][:, LHALF],
                    z_t[k].rearrange("o i kh kw -> o (i kh kw)")[:, LHALF],
                )
            for k in range(5):
                nc.sync.dma_start(
                    z_sbs[k][:, RHALF],
                    z_t[k].rearrange("o i kh kw -> o (i kh kw)")[:, RHALF],
                )

            x_tiles = [None] * n_img
            for img in range(n_img):
                x_tiles[img] = x_pool.tile(
                    [C, H * W], F32, name=f"x{img}", tag="xin"
                )
            for img in range(n_img):
                chunks = X0_CHUNKS if img == 0 else ((0, H),)
                xv = x_tiles[img].rearrange("c (h w) -> c h w", w=W)
                for r0, r1 in chunks:
                    nc.sync.dma_start(xv[:, r0:r1, :], x_t[img, :, r0:r1, :])

            identity = const_pool.tile([128, 128], BF16)
            make_identity(nc, identity)
            # rv_k * I stationaries for the synthesis matmuls (f32,
            # bitcast to f32r at use: full-rate PE at N >= 256)
            rvI = []
            rvI_f32 = const_pool.tile([128, 128], F32, name="rvI_f32")
            for k in range(5):
                nc.gpsimd.memset(rvI_f32, 0.0)
                nc.gpsimd.affine_select(
                    out=rvI_f32,
                    in_=rvI_f32,
                    compare_op=mybir.AluOpType.not_equal,
                    fill=float(rv[k]),
                    base=0,
                    pattern=[[-1, 128]],
                    channel_multiplier=1,
                )
                t = const_pool.tile([128, 128], F32R, name=f"rvI{k}")
                nc.scalar.copy(t, rvI_f32)
                rvI.append(t)

            # --- PE warm-up: keep the HAM clock gate ramping while the
            # head DMAs stream, so neither synth nor conv starts cold.
            warm_rhs = const_pool.tile([128, 448], BF16)
            nc.gpsimd.memset(warm_rhs, 0.0)
            warm_ps = cpsum_pool.tile([O, N_TILE], F32, tag="cv")
            for _ in range(N_WARM):
                nc.tensor.matmul(
                    warm_ps, identity, warm_rhs, start=True, stop=True
                )

            # --- per-image padded sign(x) buffers (borders zeroed once) ---
            padded = []
            for img in range(n_img):
                pd = img_pool.tile(
                    [C, HP2 * WP], ADT, name=f"pad{img}", tag=f"pad{img}"
                )
                pd3 = pd.rearrange("p (h w) -> p h w", w=WP)
                nc.gpsimd.memset(pd3[:, 0, 0:HP], 0.0)
                nc.gpsimd.memset(pd3[:, HP - 1, 0:HP], 0.0)
                nc.gpsimd.memset(pd3[:, 1 : HP - 1, 0:1], 0.0)
                nc.gpsimd.memset(pd3[:, 1 : HP - 1, HP - 1 : HP], 0.0)
                # dup of bottom-pad row 57 (zero); dup cols >= 56 are unread
                nc.gpsimd.memset(pd3[:, HP2 - 1, 0:HP], 0.0)
                padded.append(pd3)

            # --- weight synthesis on PE: per chunk, M -> PSUM (ACT copy),
            # then 5 f32r matmuls accumulate rv_k Z_k, sign from PSUM.
            bw_nat = wsyn_pool.tile([O, IKK], BF16)
            bw3 = bw_nat.rearrange("o (i t) -> o i t", t=NTAPS)
            syn = [
                spsum_pool.tile([O, CCH], F32, name=f"syn{i}") for i in range(2)
            ]
            for cc in range(NCHUNK):
                csl = slice(cc * CCH, (cc + 1) * CCH)
                ps = syn[cc % 2]
                nc.scalar.copy(ps, m_sb[:, csl])
                for k in range(5):
                    nc.tensor.matmul(
                        ps,
                        rvI[k],
                        z_sbs[k][:, csl],
                        start=False,
                        stop=(k == 4),
                        skip_group_check=True,
                    )
                nc.scalar.sign(bw_nat[:, csl], ps)

            # transposes: per (32-col block, tap), tile_position packs the
            # PE array; emitted per chunk so they chase the signs.
            if USE_FP8:
                bw_pairV = wsyn_pool.tile([C, KS, 2, O], FP8)
                bw_S = wsyn_pool.tile([C, KS, O], FP8)
                tpP = tpsum_pool.tile([128, KS * 2 * O], BF16)
                tpS = tpsum_pool.tile([128, KS * O], BF16)
            else:
                bw_lhsT = wsyn_pool.tile([C, NTAPS, O], BF16)
                tpP = tpsum_pool.tile([128, 4 * O], BF16)
                tpS = tpsum_pool.tile([128, 5 * O], BF16)
            for ic in range(NCHUNK):
                psl = slice(ic * 32, (ic + 1) * 32)
                for t in range(NTAPS):
                    ky, kx = divmod(t, KS)
                    if USE_FP8:
                        dst, toff = (
                            (tpP, (kx * 2 + ky) * O) if ky < 2 else (tpS, kx * O)
                        )
                    else:
                        dst, toff = (tpP, t * O) if t < 4 else (tpS, (t - 4) * O)
                    nc.tensor.transpose(
                        dst[psl, toff : toff + O],
                        bw3[:, psl, t],
                        identity,
                        tile_position=(0, ic * 32),
                    )
            if USE_FP8:
                # per-kx packs so the first conv matmul only waits for kx=0
                for kx in range(KS):
                    nc.scalar.copy(
                        bw_pairV[:, kx].rearrange("p b o -> p (b o)"),
                        tpP[:, kx * 2 * O : (kx + 1) * 2 * O],
                    )
                nc.vector.tensor_copy(
                    bw_S.rearrange("p a o -> p (a o)"), tpS
                )
            else:
                nc.vector.tensor_copy(
                    bw_lhsT[:, 0:4, :],
                    tpP.rearrange("p (t o) -> p t o", o=O),
                )
                nc.vector.tensor_copy(
                    bw_lhsT[:, 4:NTAPS, :],
                    tpS.rearrange("p (t o) -> p t o", o=O),
                )

            # --- signs: main rows + shifted dup rows, chunked + interleaved
            # on ACT so conv tiles release as x rows land.
            def sign_main_rows(img, r0, r1):
                # pd rows 1+r0 .. 1+r1 <- sign(x rows r0..r1)
                pd3 = padded[img]
                xi = x_tiles[img].rearrange("c (h w) -> c h w", w=W)
                nc.scalar.sign(
                    pd3[:, 1 + r0 : 1 + r1, 1 : 1 + W], xi[:, r0:r1, :]
                )

            def sign_dup_rows(img, r0, r1):
                # dup rows r (=pd row DUP+r), r in [r0,r1) subset of [2,57):
                # dup[r][c] = pd[r][c+1] = sign(x[r-1][c]), c < 56
                pd3 = padded[img]
                xi = x_tiles[img].rearrange("c (h w) -> c h w", w=W)
                nc.scalar.sign(
                    pd3[:, DUP + r0 : DUP + r1, 0:W],
                    xi[:, r0 - 1 : r1 - 1, :],
                )

            def sign_image(img):
                chunks = X0_CHUNKS if img == 0 else XN_CHUNKS
                for r0, r1 in chunks:
                    sign_main_rows(img, r0, r1)
                    sign_dup_rows(img, max(2, r0 + 1), min(57, r1 + 1))

            sign_image(0)

            # --- main conv loop; next image's sign emitted before this
            # image's tiles so ACT never head-of-line blocks ---
            for img in range(n_img):
                if img + 1 < n_img:
                    sign_image(img + 1)
                pd3 = padded[img]

                for nt in range(N_ROW_TILES):
                    y0 = nt * ROWS_PER_TILE
                    cv = cpsum_pool.tile([O, N_TILE], F32, tag="cv")
                    if USE_FP8:
                        # 3 vertical pairs {(0,kx),(1,kx)}, pair step WP
                        for kx in range(KS):
                            win0 = pd3[:, y0 : y0 + ROWS_PER_TILE, kx : kx + W]
                            ap4 = bass.AP(
                                win0.tensor,
                                win0.offset,
                                [list(win0.ap[0]), [WP, 2]]
                                + [list(p) for p in win0.ap[1:]],
                            )
                            nc.tensor.matmul(
                                cv,
                                bw_pairV[:, kx],
                                ap4,
                                start=(kx == 0),
                                stop=False,
                                perf_mode=mybir.MatmulPerfMode.DoubleRow,
                            )
                        # pair {(2,0),(2,1)}: elem 1 in the shifted dup rows
                        winD = pd3[:, y0 + 2 : y0 + 2 + ROWS_PER_TILE, 0:W]
                        apD = bass.AP(
                            winD.tensor,
                            winD.offset,
                            [list(winD.ap[0]), [DUP * WP, 2]]
                            + [list(p) for p in winD.ap[1:]],
                        )
                        nc.tensor.matmul(
                            cv,
                            bw_S[:, 0:2, :],
                            apD,
                            start=False,
                            stop=False,
                            perf_mode=mybir.MatmulPerfMode.DoubleRow,
                        )
                        # single tap (2,2)
                        winS = pd3[
                            :, y0 + 2 : y0 + 2 + ROWS_PER_TILE, 2 : 2 + W
                        ]
                        nc.tensor.matmul(
                            cv, bw_S[:, 2, :], winS, start=False, stop=True
                        )
                    else:
                        t = 0
                        for ky in range(KS):
                            for kx in range(KS):
                                win = pd3[
                                    :,
                                    y0 + ky : y0 + ky + ROWS_PER_TILE,
                                    kx : kx + W,
                                ]
                                nc.tensor.matmul(
                                    cv,
                                    bw_lhsT[:, t, :],
                                    win,
                                    start=(t == 0),
                                    stop=(t == NTAPS - 1),
                                )
                                t += 1
                    ev = ev_pool.tile([O, N_TILE], ODT, tag="ev")
                    nc.vector.tensor_scalar_mul(ev, cv, alpha_sb[:, 0:1])
                    nc.sync.dma_start(
                        out_t[img, :, y0 : y0 + ROWS_PER_TILE, :],
                        ev.rearrange("o (h w) -> o h w", w=W),
                    )

    nc.compile()
    return nc


def _ensure_ntff_hook():
    """Register the axon NTFF profiling hook if the image's antenv lacks it.

    Only used when BASS_KERNEL_TRACE=1 (dev profiling); best-effort.
    """
    import sys
    import types

    try:
        import antenv

        if hasattr(antenv, "axon_hooks"):
            return
        mod = types.ModuleType("antenv.axon_hooks")
        _hook = [None]
        mod.set_axon_ntff_profile_hook = lambda h: _hook.__setitem__(0, h)
        mod.get_axon_ntff_profile_hook = lambda: _hook[0]
        sys.modules["antenv.axon_hooks"] = mod
        antenv.axon_hooks = mod
        from trn_agent_boot.trn_boot import _ntff_profile_via_ctypes

        mod.set_axon_ntff_profile_hook(
            _ntff_profile_via_ctypes("/opt/axon/libaxon_pjrt.so")
        )
    except Exception as e:  # pragma: no cover - profiling is optional
        print(f"NTFF hook registration failed ({e}); tracing disabled")


def kernel(x, Alpha, M, Z, rv):
    x = np.ascontiguousarray(np.asarray(x, dtype=np.float32))
    Alpha = np.ascontiguousarray(np.asarray(Alpha, dtype=np.float32))
    M = np.ascontiguousarray(np.asarray(M, dtype=np.float32))
    Z = np.ascontiguousarray(np.asarray(Z, dtype=np.float32))
    rv = np.asarray(rv, dtype=np.float32)

    trace = bool(int(os.environ.get("BASS_KERNEL_TRACE", "0")))
    if trace:
        _ensure_ntff_hook()

    nc = build_program(rv)

    in_maps = []
    for c in range(N_CORES):
        in_maps.append(
            {
                "x": np.ascontiguousarray(x[c * B_CORE : (c + 1) * B_CORE]),
                "Alpha": Alpha,
                "M": M,
                "Z": Z,
            }
        )

    res = run_bass_kernel_spmd(
        nc,
        in_maps,
        core_ids=list(range(N_CORES)),
        trace=trace,
    )
    out = np.concatenate(
        [res.results[c]["out"] for c in range(N_CORES)], axis=0
    ).astype(np.float32)
    if trace:
        kernel.last_results = res
    return out
